# revision 1
# baseline (speedup 1.0000x reference)
"""Trainium2 Bass kernel for Points3DLoss (robust chamfer loss).

Computes, for inputs obs (2,16,4096,3) and pred (2,16,2048,3):
  d[bt,n]  = min_m |obs[bt,n] - pred[bt,m]|^2          (chamfer, per frame)
  res      = sqrt(d) reshaped to (B, T*N)
  med, mad = lower-median robust stats per batch row (on detached res)
  w        = bisquare weights; loss = 0.5 * sum(w * res^2)

Strategy: data-parallel over the 32 frames (4 per core). Each core computes
its frames' min-distances via PE matmuls (augmented K=4 dot products:
z = a.b - 0.5|b|^2, so min d = |a|^2 - 2 max z) with the row-max done by
fused DVE tensor_tensor_reduce over PSUM. A single AllGather shares the
per-frame min-distances (d, not res - sqrt is folded into the threshold
algebra); every core then redundantly computes the median/mad via bitwise
bisection on f32 bit patterns (counting passes on DVE + partition-sum via
a ones-matmul on PE) and the final weighted sum. Output read from core 0.
"""

import sys

if '/opt/trn_rl_repo' not in sys.path:
    sys.path.insert(0, '/opt/trn_rl_repo')

import numpy as np

B, T, N_OBS, M_PRED = 2, 16, 4096, 2048
BT = B * T
NCORES = 8
F = BT // NCORES          # frames per core = 4
CH = N_OBS // 128         # obs chunks per frame = 32
COLS = F * CH             # d columns per core = 128
NROW = T * N_OBS          # residuals per batch row = 65536
K_MED = float((NROW - 1) // 2 + 1)   # rank (1-based) of lower median = 32768
TUNE = 4.6851
MADSTD = 0.67449

MED_BITS = list(range(30, 8, -1))    # bisect f32 bit pattern of d, bits 30..9
MAD_BITS = list(range(30, 11, -1))   # bisect f32 bit pattern of t,  bits 30..12

_CACHE = {}


def _build_nc(stage="D", reps=1):
    """stage: A=main loop only, B=+allgather, C=+median, D=full kernel."""
    import concourse.bacc as bacc
    import concourse.tile as tile
    from concourse import mybir
    from contextlib import ExitStack

    A = mybir.AluOpType
    AF = mybir.ActivationFunctionType
    f32 = mybir.dt.float32
    u32 = mybir.dt.uint32
    X = mybir.AxisListType.X

    nc = bacc.Bacc("TRN2", target_bir_lowering=False, debug=False,
                   num_devices=NCORES)

    obs_t = nc.dram_tensor("obs_t", [4, F * N_OBS], f32, kind="ExternalInput").ap()
    pred_t = nc.dram_tensor("pred_t", [4, F * M_PRED], f32, kind="ExternalInput").ap()
    obs_sq = nc.dram_tensor("obs_sq", [128, COLS * 3], f32, kind="ExternalInput").ap()
    out_d = nc.dram_tensor("out", [1, 1], f32, kind="ExternalOutput").ap()
    dbg = None
    if stage in ("A", "B", "C"):
        dbg = nc.dram_tensor("dbg", [128, NCORES * COLS], f32,
                             kind="ExternalOutput").ap()

    def emit(tc, pp):

        OBS = pp.tile([4, F * N_OBS], f32, name="OBS", tag="OBS")
        PRED = pp.tile([4, F * M_PRED], f32, name="PRED", tag="PRED")
        OSQ_SRC = pp.tile([128, COLS * 3], f32, name="OSQ_SRC", tag="OSQ_SRC")
        nc.sync.dma_start(out=OBS, in_=obs_t)
        nc.sync.dma_start(out=PRED, in_=pred_t)
        nc.sync.dma_start(out=OSQ_SRC, in_=obs_sq)

        # |a|^2 per obs point, laid out [p, (f c)] to match zmax columns
        osq = pp.tile([128, COLS * 3], f32, name="osq", tag="osq")
        onorm = pp.tile([128, COLS], f32, name="onorm", tag="onorm")
        nc.scalar.activation(out=osq, in_=OSQ_SRC, func=AF.Square)
        nc.vector.tensor_reduce(
            out=onorm, in_=osq.rearrange("p (c d) -> p c d", d=3), axis=X, op=A.add)

        # lhsT for pred-norm matmul: contract rows 1-3 (coords) with -0.5,
        # ignore row 0 (holds garbage squares of the evolving norm row)
        neg_half = pp.tile([4, 128], f32, name="neg_half", tag="neg_half")
        nc.vector.memset(neg_half, -0.5)
        nc.vector.memset(neg_half[0:1, :], 0.0)
        ones128 = pp.tile([128, 128], f32, name="ones128", tag="ones128")
        nc.vector.memset(ones128, 1.0)
        negones = pp.tile([128, 128], f32, name="negones", tag="negones")
        nc.vector.memset(negones, -1.0)
        half1 = pp.tile([128, 1], f32, name="half1", tag="half1")
        nc.vector.memset(half1, 0.5)

        # --- prep: PRED row 0 = -0.5 * |b|^2 (per frame) ---------------------
        with tc.tile_pool(name="prep_ps", bufs=2, space="PSUM") as auxp, \
             tc.tile_pool(name="prep_sb", bufs=2) as sqp:
            for f in range(F):
                sq = sqp.tile([4, M_PRED], f32, name=f"sq{f}", tag="sq")
                nc.scalar.activation(
                    out=sq, in_=PRED[0:4, f * M_PRED:(f + 1) * M_PRED],
                    func=AF.Square)
                for q in range(4):
                    pn = auxp.tile([128, 512], f32, name=f"pn{f}_{q}", tag="pn")
                    nc.tensor.matmul(pn, lhsT=neg_half,
                                     rhs=sq[:, q * 512:(q + 1) * 512],
                                     start=True, stop=True)
                    lo = f * M_PRED + q * 512
                    nc.scalar.copy(out=PRED[0:1, lo:lo + 512], in_=pn[0:1, :])

        # --- main: z[n,m] = a.b - 0.5|b|^2 ; zmax = max_m z ------------------
        # Row-max split for engine balance: DVE max-reduces PSUM bank 0
        # directly (1x from PSUM), ACT stages banks 1-3 to SBUF where DVE
        # max-reduces at 2x, chaining the partial via the scalar2 init.
        zmax = pp.tile([128, COLS], f32, name="zmax", tag="zmax")
        junk = pp.tile([128, 1536], f32, name="junk", tag="junk")
        ztmp = pp.tile([128, 1], f32, name="ztmp", tag="ztmp")
        with tc.tile_pool(name="mm", bufs=2, space="PSUM") as mmp, \
             tc.tile_pool(name="cpyp", bufs=3) as cpyp:
            for f in range(F):
                for c in range(CH):
                    ps = mmp.tile([128, M_PRED], f32, name="mmps", tag="mmps")
                    lhsT = OBS[:, f * N_OBS + c * 128: f * N_OBS + (c + 1) * 128]
                    for q in range(4):
                        nc.tensor.matmul(
                            ps[:, q * 512:(q + 1) * 512], lhsT=lhsT,
                            rhs=PRED[:, f * M_PRED + q * 512: f * M_PRED + (q + 1) * 512],
                            start=True, stop=True)
                    cpy = cpyp.tile([128, 1536], f32, name="cpy", tag="cpy")
                    nc.scalar.copy(out=cpy, in_=ps[:, 512:2048])
                    nc.vector.tensor_scalar(
                        out=junk[:, 0:512], in0=ps[:, 0:512], scalar1=-1e30,
                        scalar2=None, op0=A.max, op1=A.max, accum_out=ztmp)
                    nc.vector.tensor_scalar(
                        out=junk, in0=cpy, scalar1=-1e30, scalar2=ztmp,
                        op0=A.max, op1=A.max,
                        accum_out=zmax[:, f * CH + c: f * CH + c + 1])

        # d = max(|a|^2 - 2*zmax, 0)
        d_all = pp.tile([128, COLS], f32, name="d_all", tag="d_all")
        nc.vector.scalar_tensor_tensor(
            out=d_all, in0=zmax, scalar=-2.0, op0=A.mult, op1=A.add, in1=onorm)
        nc.vector.tensor_scalar_max(d_all, d_all, 0.0)

        if stage == "A":
            nc.sync.dma_start(out=dbg[:, 0:COLS], in_=d_all)
            return

        # --- allgather d across the 8 cores ---------------------------------
        if stage == "T":
            # timeline-sim variant: no collective (single-core cost model);
            # fake the gather by replicating local d
            g = pp.tile([128, NCORES * COLS], f32, name="g", tag="g")
            for r in range(NCORES):
                nc.scalar.copy(out=g[:, r * COLS:(r + 1) * COLS], in_=d_all)
        else:
            with tc.tile_pool(name="dram", bufs=1, space="DRAM") as dp:
                cc_in = dp.tile([128, COLS], f32, name="cc_in")
                cc_out = dp.tile([NCORES, 128, COLS], f32, name="cc_out",
                                 addr_space="Shared")
                nc.sync.dma_start(out=cc_in, in_=d_all)
                nc.gpsimd.collective_compute(
                    "AllGather", A.bypass,
                    replica_groups=[list(range(NCORES))],
                    ins=[cc_in[:]], outs=[cc_out[:]])
                g = pp.tile([128, NCORES * COLS], f32, name="g", tag="g")
                nc.sync.dma_start(
                    out=g.rearrange("p (r c) -> p r c", r=NCORES),
                    in_=cc_out.rearrange("r p c -> p r c"))

        if stage == "B":
            nc.sync.dma_start(out=dbg, in_=g)
            return

        d0 = g[:, 0:512]      # batch row 0 (cores 0-3)
        d1 = g[:, 512:1024]   # batch row 1 (cores 4-7)

        cnt2 = pp.tile([128, 2], f32, name="cnt2", tag="cnt2")
        delta = pp.tile([128, 2], f32, name="delta", tag="delta")
        jk = junk[:, 0:512]

        with tc.tile_pool(name="bis_ps", bufs=2, space="PSUM") as bp:

            def bisect_med():
                Ts = pp.tile([128, 2], f32, name="Ts_med", tag="Ts_med")
                nc.vector.memset(Ts, float(2 ** 21))
                Tu = pp.tile([128, 2], u32, name="Tu_med", tag="Tu_med")
                nc.vector.tensor_scalar(out=Tu, in0=Ts, scalar1=512.0,
                                        scalar2=None, op0=A.mult)
                Tf = Tu.bitcast(f32)
                for j in MED_BITS:
                    nc.vector.tensor_scalar(
                        out=jk, in0=d0, scalar1=Tf[:, 0:1], scalar2=None,
                        op0=A.is_lt, op1=A.add, accum_out=cnt2[:, 0:1])
                    nc.vector.tensor_scalar(
                        out=jk, in0=d1, scalar1=Tf[:, 1:2], scalar2=None,
                        op0=A.is_lt, op1=A.add, accum_out=cnt2[:, 1:2])
                    tot = bp.tile([128, 2], f32, name="tot_med", tag="tot")
                    nc.tensor.matmul(tot, lhsT=ones128, rhs=cnt2,
                                     start=True, stop=True)
                    nc.vector.tensor_scalar(
                        out=delta, in0=tot, scalar1=K_MED,
                        scalar2=float(2 ** (j - 9)), op0=A.is_lt, op1=A.mult)
                    nc.vector.scalar_tensor_tensor(
                        out=Ts, in0=delta, scalar=float(2 ** (j - 10)),
                        op0=A.subtract, op1=A.add, in1=Ts)
                    nc.vector.tensor_scalar(out=Tu, in0=Ts, scalar1=512.0,
                                            scalar2=None, op0=A.mult)
                return Tf  # center-of-bracket estimate of median(d) per row

            med_d = bisect_med()
            med = pp.tile([128, 2], f32, name="med", tag="med")
            nc.scalar.activation(out=med, in_=med_d, func=AF.Sqrt)

            if stage == "C":
                nc.sync.dma_start(out=dbg[:, 0:2], in_=med_d)
                nc.sync.dma_start(out=dbg[:, 2:4], in_=med)
                return

            def bisect_mad():
                Ts = pp.tile([128, 2], f32, name="Ts_mad", tag="Ts_mad")
                nc.vector.memset(Ts, float(2 ** 21))
                Tu = pp.tile([128, 2], u32, name="Tu_mad", tag="Tu_mad")
                nc.vector.tensor_scalar(out=Tu, in0=Ts, scalar1=512.0,
                                        scalar2=None, op0=A.mult)
                Tf = Tu.bitcast(f32)
                splus = pp.tile([128, 2], f32, name="splus", tag="splus")
                sminus = pp.tile([128, 2], f32, name="sminus", tag="sminus")
                a2 = pp.tile([128, 2], f32, name="a2", tag="a2")
                b2 = pp.tile([128, 2], f32, name="b2", tag="b2")
                cnta = pp.tile([128, 2], f32, name="cnta", tag="cnta")
                cntb = pp.tile([128, 2], f32, name="cntb", tag="cntb")
                for j in MAD_BITS:
                    # band thresholds in d-domain: a=(med+t)^2, b=max(med-t,0)^2
                    nc.vector.tensor_tensor(out=splus, in0=med, in1=Tf, op=A.add)
                    nc.vector.tensor_tensor(out=sminus, in0=med, in1=Tf,
                                            op=A.subtract)
                    nc.vector.tensor_scalar_max(sminus, sminus, 0.0)
                    nc.vector.tensor_tensor(out=a2, in0=splus, in1=splus,
                                            op=A.mult)
                    nc.vector.tensor_tensor(out=b2, in0=sminus, in1=sminus,
                                            op=A.mult)
                    for r, dr in ((0, d0), (1, d1)):
                        nc.vector.tensor_scalar(
                            out=jk, in0=dr, scalar1=a2[:, r:r + 1],
                            scalar2=None, op0=A.is_le, op1=A.add,
                            accum_out=cnta[:, r:r + 1])
                        nc.vector.tensor_scalar(
                            out=jk, in0=dr, scalar1=b2[:, r:r + 1],
                            scalar2=None, op0=A.is_lt, op1=A.add,
                            accum_out=cntb[:, r:r + 1])
                    tot = bp.tile([128, 2], f32, name="tot_mad", tag="tot")
                    nc.tensor.matmul(tot, lhsT=ones128, rhs=cnta,
                                     start=True, stop=False)
                    nc.tensor.matmul(tot, lhsT=negones, rhs=cntb,
                                     start=False, stop=True)
                    nc.vector.tensor_scalar(
                        out=delta, in0=tot, scalar1=K_MED,
                        scalar2=float(2 ** (j - 9)), op0=A.is_lt, op1=A.mult)
                    nc.vector.scalar_tensor_tensor(
                        out=Ts, in0=delta, scalar=float(2 ** (j - 10)),
                        op0=A.subtract, op1=A.add, in1=Ts)
                    nc.vector.tensor_scalar(out=Tu, in0=Ts, scalar1=512.0,
                                            scalar2=None, op0=A.mult)
                return Tf  # mad estimate (res domain) per row

            mad = bisect_mad()

            # --- loss = 0.5 * sum over rows of sum(w * d),
            #     w = relu(1 - d/(TUNE*std)^2)^2, std = mad/MADSTD ------------
            c1 = pp.tile([128, 2], f32, name="c1", tag="c1")
            nc.vector.tensor_scalar(out=c1, in0=mad, scalar1=TUNE / MADSTD,
                                    scalar2=None, op0=A.mult)
            cs2 = pp.tile([128, 2], f32, name="cs2", tag="cs2")
            nc.vector.tensor_tensor(out=cs2, in0=c1, in1=c1, op=A.mult)
            inv = pp.tile([128, 2], f32, name="inv", tag="inv")
            nc.vector.reciprocal(inv, cs2)

            S = pp.tile([128, 2], f32, name="S", tag="S")
            v = pp.tile([128, 512], f32, name="v", tag="v")
            y = pp.tile([128, 512], f32, name="y", tag="y")
            for r, dr in ((0, d0), (1, d1)):
                nc.vector.tensor_scalar(out=jk, in0=dr,
                                        scalar1=inv[:, r:r + 1], scalar2=None,
                                        op0=A.mult)
                nc.scalar.activation(out=v, in_=jk, func=AF.Relu,
                                     bias=1.0, scale=-1.0)
                nc.vector.tensor_tensor(out=y, in0=v, in1=dr, op=A.mult)
                nc.vector.scalar_tensor_tensor(
                    out=jk, in0=y, scalar=1.0, op0=A.bypass, op1=A.mult,
                    in1=v, accum_out=S[:, r:r + 1])

            ls = bp.tile([1, 2], f32, name="ls")
            nc.tensor.matmul(ls, lhsT=half1, rhs=S, start=True, stop=True)
            ls_sb = pp.tile([1, 2], f32, name="ls_sb", tag="ls_sb")
            nc.scalar.copy(out=ls_sb, in_=ls)
            lt = pp.tile([1, 1], f32, name="lt", tag="lt")
            nc.vector.tensor_tensor(out=lt, in0=ls_sb[0:1, 0:1],
                                    in1=ls_sb[0:1, 1:2], op=A.add)
            nc.sync.dma_start(out=out_d, in_=lt)

    from contextlib import ExitStack
    with tile.TileContext(nc) as tc, ExitStack() as stack:
        pp = stack.enter_context(tc.tile_pool(name="persist", bufs=1))
        for _rep in range(reps):
            emit(tc, pp)

    nc.compile()
    return nc


def _shard_inputs(points3d_obs, points3d_pred):
    obs = np.asarray(points3d_obs, dtype=np.float32).reshape(BT, N_OBS, 3)
    pred = np.asarray(points3d_pred, dtype=np.float32).reshape(BT, M_PRED, 3)
    in_maps = []
    for core in range(NCORES):
        so = obs[core * F:(core + 1) * F]       # [F, N, 3]
        sp = pred[core * F:(core + 1) * F]      # [F, M, 3]
        obs_t = np.concatenate(
            [np.ones((1, F * N_OBS), np.float32),
             so.transpose(2, 0, 1).reshape(3, F * N_OBS)], axis=0)
        pred_t = np.concatenate(
            [np.zeros((1, F * M_PRED), np.float32),
             sp.transpose(2, 0, 1).reshape(3, F * M_PRED)], axis=0)
        obs_sq = np.ascontiguousarray(
            so.reshape(F, CH, 128, 3).transpose(2, 0, 1, 3).reshape(128, COLS * 3))
        in_maps.append({
            "obs_t": np.ascontiguousarray(obs_t),
            "pred_t": np.ascontiguousarray(pred_t),
            "obs_sq": obs_sq,
        })
    return in_maps


def _get_nc(stage="D", reps=1):
    key = f"nc_{stage}_{reps}"
    if key not in _CACHE:
        _CACHE[key] = _build_nc(stage, reps)
    return _CACHE[key]


def run(points3d_obs, points3d_pred, stage="D", **kwargs):
    """Run on hardware; kwargs forwarded to run_bass_kernel_spmd (e.g. trace)."""
    from concourse.bass_utils import run_bass_kernel_spmd
    nc = _get_nc(stage)
    in_maps = _shard_inputs(points3d_obs, points3d_pred)
    res = run_bass_kernel_spmd(nc, in_maps, list(range(NCORES)), **kwargs)
    return res


def kernel(points3d_obs, points3d_pred):
    res = run(points3d_obs, points3d_pred)
    loss = np.float32(res.results[0]["out"][0, 0])
    return np.asarray(loss, dtype=np.float32).reshape(())



# revision 19
# speedup vs baseline: 3.1383x; 3.1383x over previous
"""Trainium2 Bass kernel for Points3DLoss (robust chamfer loss) — v3.

Math: for obs (2,16,4096,3), pred (2,16,2048,3):
  d[bt,n] = min_m |obs[bt,n]-pred[bt,m]|^2 ; res = sqrt(d) as (B, T*N)
  med/mad = lower medians per batch row; bisquare weights; loss = 0.5*sum(w d).

Implementation (data-parallel over 32 frames, 4/core):
  * PE computes w[n,m] = a.b - 0.5|a|^2 - 0.5|b|^2 = -d/2 via ONE bf16 matmul
    with split-precision (hi+lo) inputs packed along K=13 (error ~1e-5), so
    PSUM values near the per-row max are tiny and fp16-safe.
  * Per 128-obs chunk: ACT converts psa [128,1536] PSUM to fp16 SBUF, DVE
    max-scans psb [128,512] from PSUM directly, then runs a batched
    pairwise-max tree (fp16 tensor_tensor at 2x) over 4 chunks' fp16 data.
  * d = max(-2*zmax, 0) in fp16; AllGather in two fp16 halves (the first
    overlaps the second half of the main loop).
  * med/mad: radix-4 bisection on fp16 bit patterns (5 phases, counts via
    DVE is_lt scans + fp16 ones-matmul partition sum) to a 32-pattern
    bracket, then exact-count linear interpolation.
  * bisquare weights + weighted sum on f32-converted d; output from core 0.
"""

import sys

if '/opt/trn_rl_repo' not in sys.path:
    sys.path.insert(0, '/opt/trn_rl_repo')

import numpy as np

B, T, N_OBS, M_PRED = 2, 16, 4096, 2048
BT = B * T
NCORES = 8
F = BT // NCORES          # frames per core = 4
CH = N_OBS // 128         # obs chunks per frame = 32
COLS = F * CH             # chunks (and d columns) per core = 128
NROW = T * N_OBS          # residuals per batch row = 65536
K_MED = float((NROW - 1) // 2 + 1)   # 1-based rank of lower median = 32768
TUNE = 4.6851
MADSTD = 0.67449

# main-loop reduce split per 2048-wide chunk
ACT_N = 1536
PSB_N = 512
CVW = ACT_N              # fp16 elems per chunk in cv
SUPER = 4                # chunks per DVE tree batch
PHASES = 5               # radix-4 bisection phases before interpolation
W0 = float(2 ** 15)      # initial bracket width in fp16-pattern units
STEPS = [W0 / 4.0 ** (p + 1) for p in range(PHASES)]
WF = STEPS[-1]           # final bracket width = 32 patterns
HCOL = COLS // 2         # columns per gather half = 64

_CACHE = {}
DEBUG = False


def _build_nc():
    import concourse.bacc as bacc
    import concourse.tile as tile
    from concourse import mybir
    from contextlib import ExitStack

    A = mybir.AluOpType
    AF = mybir.ActivationFunctionType
    f32 = mybir.dt.float32
    f16 = mybir.dt.float16
    bf16 = mybir.dt.bfloat16
    u16 = mybir.dt.uint16
    X = mybir.AxisListType.X

    nc = bacc.Bacc("TRN2", target_bir_lowering=False, debug=False,
                   num_devices=NCORES)

    obs_t = nc.dram_tensor("obs_t", [13, F * N_OBS], bf16,
                           kind="ExternalInput").ap()
    pred_t = nc.dram_tensor("pred_t", [13, F * M_PRED], bf16,
                            kind="ExternalInput").ap()
    stepv_t = nc.dram_tensor("stepv_t", [128, PHASES * 6], f32,
                             kind="ExternalInput").ap()
    out_d = nc.dram_tensor("out", [1, 1], f32, kind="ExternalOutput").ap()
    dbg = None
    if DEBUG:
        dbg = nc.dram_tensor("dbg", [128, 1024 + 160], f32,
                             kind="ExternalOutput").ap()

    with tile.TileContext(nc) as tc, ExitStack() as stack:
        pp = stack.enter_context(tc.tile_pool(name="persist", bufs=1))

        OBS = pp.tile([13, F * N_OBS], bf16, name="OBS", tag="OBS")
        PRED = pp.tile([13, F * M_PRED], bf16, name="PRED", tag="PRED")
        STEPV = pp.tile([128, PHASES * 6], f32, name="STEPV", tag="STEPV")
        nc.sync.dma_start(out=OBS, in_=obs_t)
        nc.sync.dma_start(out=PRED, in_=pred_t)
        nc.sync.dma_start(out=STEPV, in_=stepv_t)

        zmax = pp.tile([128, COLS], f32, name="zmax", tag="zmax")
        jk = pp.tile([128, 512], f32, name="jk", tag="jk")
        ones16 = pp.tile([128, 128], f16, name="ones16", tag="ones16")
        nc.vector.memset(ones16, 1.0)
        half1 = pp.tile([128, 1], f32, name="half1", tag="half1")
        nc.vector.memset(half1, 0.5)

        g16 = pp.tile([128, NCORES * COLS], f16, name="g16", tag="g16")
        d16a = pp.tile([128, HCOL], f16, name="d16a", tag="d16a")
        d16b = pp.tile([128, HCOL], f16, name="d16b", tag="d16b")

        dram = stack.enter_context(
            tc.tile_pool(name="dram", bufs=1, space="DRAM"))
        cc_in_a = dram.tile([128, HCOL], f16, name="cc_in_a")
        cc_in_b = dram.tile([128, HCOL], f16, name="cc_in_b")
        cc_out_a = dram.tile([NCORES, 128, HCOL], f16, name="cc_out_a",
                             addr_space="Shared")
        cc_out_b = dram.tile([NCORES, 128, HCOL], f16, name="cc_out_b",
                             addr_space="Shared")

        def gather_half(d16, cc_in, cc_out, zlo):
            nc.vector.tensor_scalar(
                out=d16, in0=zmax[:, zlo:zlo + HCOL], scalar1=-2.0,
                scalar2=0.0, op0=A.mult, op1=A.max)
            nc.sync.dma_start(out=cc_in, in_=d16)
            nc.gpsimd.collective_compute(
                "AllGather", A.bypass,
                replica_groups=[list(range(NCORES))],
                ins=[cc_in[:]], outs=[cc_out[:]])
            # row0 (cores 0-3) -> g16[:, zlo*4 : zlo*4+256]
            # row1 (cores 4-7) -> g16[:, 512+zlo*4 : 512+zlo*4+256]
            for half, base in ((0, zlo * 4), (1, 512 + zlo * 4)):
                nc.sync.dma_start(
                    out=g16[:, base:base + 4 * HCOL].rearrange(
                        "p (r c) -> p r c", r=4),
                    in_=cc_out[4 * half:4 * half + 4].rearrange(
                        "r p c -> p r c"))

        # ---------------- main loop: w = -d/2, chunk-max ------------------
        with tc.tile_pool(name="psa", bufs=2, space="PSUM") as psap, \
             tc.tile_pool(name="psb", bufs=2, space="PSUM") as psbp, \
             tc.tile_pool(name="cvp", bufs=2) as cvp, \
             tc.tile_pool(name="zddp", bufs=2) as zddp:
            for sc in range(COLS // SUPER):
                cv = cvp.tile([128, SUPER * CVW], f16, name=f"cv{sc}",
                              tag="cv")
                zdd = zddp.tile([128, SUPER], f32, name=f"zdd{sc}", tag="zdd")
                for j in range(SUPER):
                    col = sc * SUPER + j
                    f = col // CH
                    lhsT = OBS[:, col * 128:(col + 1) * 128]
                    psa = psap.tile([128, ACT_N], f32, name="psa", tag="psa")
                    psb = psbp.tile([128, PSB_N], f32, name="psb", tag="psb")
                    for q in range(3):
                        nc.tensor.matmul(
                            psa[:, q * 512:(q + 1) * 512], lhsT=lhsT,
                            rhs=PRED[:, f * M_PRED + q * 512:
                                     f * M_PRED + (q + 1) * 512],
                            start=True, stop=True)
                    nc.tensor.matmul(
                        psb, lhsT=lhsT,
                        rhs=PRED[:, f * M_PRED + 3 * 512:f * M_PRED + 2048],
                        start=True, stop=True)
                    nc.scalar.activation(
                        out=cv[:, j * CVW:j * CVW + ACT_N],
                        in_=psa, func=AF.Copy)
                    nc.vector.tensor_scalar(
                        out=jk, in0=psb, scalar1=-1e30, scalar2=None,
                        op0=A.max, op1=A.max, accum_out=zdd[:, j:j + 1])
                # DVE fp16 pairwise-max tree over [128, SUPER, CVW]
                v = cv.rearrange("p (c n) -> p c n", c=SUPER)
                w = CVW // 2
                while w >= 96:
                    nc.vector.tensor_tensor(
                        out=v[:, :, 0:w], in0=v[:, :, 0:w],
                        in1=v[:, :, w:2 * w], op=A.max)
                    w //= 2
                zt4 = zddp.tile([128, SUPER], f32, name=f"zt{sc}", tag="zt")
                nc.vector.tensor_reduce(
                    out=zt4, in_=v[:, :, 0:2 * w], axis=X, op=A.max)
                nc.vector.tensor_tensor(
                    out=zmax[:, sc * SUPER:(sc + 1) * SUPER], in0=zt4,
                    in1=zdd, op=A.max)
                if sc == (COLS // SUPER) // 2 - 1:
                    gather_half(d16a, cc_in_a, cc_out_a, 0)
            gather_half(d16b, cc_in_b, cc_out_b, HCOL)

        r0 = g16[:, 0:512]      # batch row 0 (cores 0-3)
        r1 = g16[:, 512:1024]   # batch row 1 (cores 4-7)

        # f32 copy of d for the weighted sum; sqrt for the mad domain.
        gf32 = pp.tile([128, 1024], f32, name="gf32", tag="gf32")
        nc.scalar.activation(out=gf32, in_=g16, func=AF.Copy)
        rs16 = pp.tile([128, 1024], f16, name="rs16", tag="rs16")
        nc.scalar.activation(out=rs16, in_=g16, func=AF.Sqrt)

        # ---------------- median via pattern bisection + interp -----------
        with tc.tile_pool(name="bis_ps", bufs=2, space="PSUM") as bp:

            def bisect(r0_, r1_, name):
                """Interpolated K_MED-th smallest of each 65536-value fp16
                row, via radix-4 bisection on fp16 bit patterns."""
                lo = pp.tile([128, 2], f32, name=f"lo_{name}", tag=f"lo{name}")
                nc.vector.memset(lo, 0.0)
                cand = pp.tile([128, 6], f32, name=f"cand_{name}",
                               tag=f"ca{name}")
                Tu = pp.tile([128, 6], u16, name=f"Tu_{name}", tag=f"Tu{name}")
                cnt = pp.tile([128, 6], f32, name=f"cnt_{name}",
                              tag=f"cn{name}")
                cnt16 = pp.tile([128, 6], f16, name=f"cnt16_{name}",
                                tag=f"c6{name}")
                sel = pp.tile([128, 6], f32, name=f"sel_{name}",
                              tag=f"se{name}")
                nsel = pp.tile([128, 2], f32, name=f"nsel_{name}",
                               tag=f"ns{name}")
                Tv = pp.tile([128, 6], f32, name=f"Tv_{name}",
                             tag=f"Tv{name}")
                for p in range(PHASES):
                    step = STEPS[p]
                    # col r*3+i holds lo[r] + (i+1)*step via host constants
                    for r in range(2):
                        nc.vector.tensor_scalar(
                            out=cand[:, r * 3:r * 3 + 3],
                            in0=STEPV[:, p * 6 + r * 3:p * 6 + r * 3 + 3],
                            scalar1=lo[:, r:r + 1], scalar2=None, op0=A.add)
                    nc.vector.tensor_scalar(
                        out=Tu, in0=cand, scalar1=1.0, scalar2=None,
                        op0=A.mult)
                    nc.vector.tensor_copy(out=Tv, in_=Tu.bitcast(f16))
                    for r, dr in ((0, r0_), (1, r1_)):
                        for i in range(3):
                            nc.vector.tensor_scalar(
                                out=jk, in0=dr, scalar1=Tv[:, r * 3 + i:
                                                           r * 3 + i + 1],
                                scalar2=None, op0=A.is_lt, op1=A.add,
                                accum_out=cnt[:, r * 3 + i:r * 3 + i + 1])
                    nc.vector.tensor_scalar(
                        out=cnt16, in0=cnt, scalar1=1.0, scalar2=None,
                        op0=A.mult)
                    tot = bp.tile([128, 6], f32, name=f"tot{name}{p}",
                                  tag="tot")
                    nc.tensor.matmul(tot, lhsT=ones16, rhs=cnt16,
                                     start=True, stop=True)
                    nc.vector.tensor_scalar(
                        out=sel, in0=tot, scalar1=K_MED, scalar2=None,
                        op0=A.is_lt)
                    nc.vector.tensor_reduce(
                        out=nsel, in_=sel.rearrange("p (r i) -> p r i", r=2),
                        axis=X, op=A.add)
                    nc.vector.scalar_tensor_tensor(
                        out=lo, in0=nsel, scalar=float(step), op0=A.mult,
                        op1=A.add, in1=lo)
                    if DEBUG and name == "med":
                        base = 1040 + p * 20
                        nc.sync.dma_start(out=dbg[:, base:base + 6], in_=cnt)
                        nc.sync.dma_start(out=dbg[:, base + 6:base + 12],
                                          in_=sel)
                        nc.sync.dma_start(out=dbg[:, base + 12:base + 14],
                                          in_=nsel)
                        nc.sync.dma_start(out=dbg[:, base + 14:base + 16],
                                          in_=lo)
                # final: counts at V0=lo, V1=lo+WF, then interpolate
                nc.vector.tensor_scalar(
                    out=cand[:, 0:2], in0=lo, scalar1=0.0, scalar2=None,
                    op0=A.add)
                nc.vector.tensor_scalar(
                    out=cand[:, 2:4], in0=lo, scalar1=float(WF), scalar2=None,
                    op0=A.add)
                nc.vector.tensor_scalar(
                    out=Tu[:, 0:4], in0=cand[:, 0:4], scalar1=1.0,
                    scalar2=None, op0=A.mult)
                nc.vector.tensor_copy(out=Tv[:, 0:4],
                                      in_=Tu.bitcast(f16)[:, 0:4])
                for r, dr in ((0, r0_), (1, r1_)):
                    for i in range(2):
                        nc.vector.tensor_scalar(
                            out=jk, in0=dr,
                            scalar1=Tv[:, 2 * i + r:2 * i + r + 1],
                            scalar2=None, op0=A.is_lt, op1=A.add,
                            accum_out=cnt[:, 2 * i + r:2 * i + r + 1])
                nc.vector.tensor_scalar(
                    out=cnt16[:, 0:4], in0=cnt[:, 0:4], scalar1=1.0,
                    scalar2=None, op0=A.mult)
                tot = bp.tile([128, 4], f32, name=f"totF{name}", tag="tot")
                nc.tensor.matmul(tot, lhsT=ones16, rhs=cnt16[:, 0:4],
                                 start=True, stop=True)
                Csb = pp.tile([128, 4], f32, name=f"Csb_{name}",
                              tag=f"Cs{name}")
                nc.vector.tensor_copy(out=Csb, in_=tot)
                C0 = Csb[:, 0:2]
                C1 = Csb[:, 2:4]
                V0 = Tv[:, 0:2]
                V1 = Tv[:, 2:4]
                den = pp.tile([128, 2], f32, name=f"den_{name}",
                              tag=f"de{name}")
                nc.vector.tensor_tensor(out=den, in0=C1, in1=C0,
                                        op=A.subtract)
                rec = pp.tile([128, 2], f32, name=f"rec_{name}",
                              tag=f"re{name}")
                nc.vector.reciprocal(rec, den)
                # scalar_tensor_tensor computes (in0 op0 scalar) op1 in1 on
                # HW, so build med = V0 - dV*((C0-K)*rec) with dV = V1-V0.
                frac = pp.tile([128, 2], f32, name=f"frac_{name}",
                               tag=f"fr{name}")
                nc.vector.scalar_tensor_tensor(
                    out=frac, in0=C0, scalar=K_MED, op0=A.subtract,
                    op1=A.mult, in1=rec)          # (C0-K)/(C1-C0) <= 0
                dV = pp.tile([128, 2], f32, name=f"dV_{name}",
                             tag=f"dV{name}")
                nc.vector.tensor_tensor(out=dV, in0=V1, in1=V0,
                                        op=A.subtract)
                med = pp.tile([128, 2], f32, name=f"med_{name}",
                              tag=f"md{name}")
                nc.vector.tensor_tensor(out=med, in0=dV, in1=frac,
                                        op=A.mult)  # dV*frac <= 0
                nc.vector.tensor_tensor(out=med, in0=V0, in1=med,
                                        op=A.subtract)
                return med

            med_d = bisect(r0, r1, "med")      # median of d per row

            # residual domain: med_r = sqrt(med_d); t = |sqrt(d) - med_r|
            med_r = pp.tile([128, 2], f32, name="med_r", tag="med_r")
            nc.scalar.activation(out=med_r, in_=med_d, func=AF.Sqrt)
            t16 = pp.tile([128, 1024], f16, name="t16", tag="t16")
            for r in range(2):
                nc.vector.tensor_scalar(
                    out=t16[:, r * 512:(r + 1) * 512],
                    in0=rs16[:, r * 512:(r + 1) * 512],
                    scalar1=med_r[:, r:r + 1], scalar2=None,
                    op0=A.subtract)
            nc.scalar.activation(out=t16, in_=t16, func=AF.Abs)

            mad = bisect(t16[:, 0:512], t16[:, 512:1024], "mad")

            # ---------------- bisquare weights + loss ---------------------
            c1 = pp.tile([128, 2], f32, name="c1", tag="c1")
            nc.vector.tensor_scalar(out=c1, in0=mad, scalar1=TUNE / MADSTD,
                                    scalar2=None, op0=A.mult)
            cs2 = pp.tile([128, 2], f32, name="cs2", tag="cs2")
            nc.vector.tensor_tensor(out=cs2, in0=c1, in1=c1, op=A.mult)
            inv = pp.tile([128, 2], f32, name="inv", tag="inv")
            nc.vector.reciprocal(inv, cs2)

            S = pp.tile([128, 2], f32, name="S", tag="S")
            vv = pp.tile([128, 512], f32, name="vv", tag="vv")
            y = pp.tile([128, 512], f32, name="y", tag="y")
            for r in range(2):
                dr = gf32[:, r * 512:(r + 1) * 512]
                nc.vector.tensor_scalar(out=jk, in0=dr,
                                        scalar1=inv[:, r:r + 1],
                                        scalar2=None, op0=A.mult)
                nc.scalar.activation(out=vv, in_=jk, func=AF.Relu,
                                     bias=1.0, scale=-1.0)
                nc.vector.tensor_tensor(out=y, in0=vv, in1=dr, op=A.mult)
                nc.vector.scalar_tensor_tensor(
                    out=jk, in0=y, scalar=1.0, op0=A.bypass, op1=A.mult,
                    in1=vv, accum_out=S[:, r:r + 1])

            if DEBUG:
                nc.sync.dma_start(out=dbg[:, 0:1024], in_=gf32)
                nc.sync.dma_start(out=dbg[:, 1024:1026], in_=med_d)
                nc.sync.dma_start(out=dbg[:, 1026:1028], in_=med_r)
                nc.sync.dma_start(out=dbg[:, 1028:1030], in_=mad)
                nc.sync.dma_start(out=dbg[:, 1030:1032], in_=inv)
                nc.sync.dma_start(out=dbg[:, 1032:1034], in_=S)
                nc.sync.dma_start(out=dbg[:, 1034:1036], in_=c1)

            ls = bp.tile([1, 2], f32, name="ls")
            nc.tensor.matmul(ls, lhsT=half1, rhs=S, start=True, stop=True)
            ls_sb = pp.tile([1, 2], f32, name="ls_sb", tag="ls_sb")
            nc.scalar.copy(out=ls_sb, in_=ls)
            lt = pp.tile([1, 1], f32, name="lt", tag="lt")
            nc.vector.tensor_tensor(out=lt, in0=ls_sb[0:1, 0:1],
                                    in1=ls_sb[0:1, 1:2], op=A.add)
            nc.sync.dma_start(out=out_d, in_=lt)

    nc.compile()
    return nc


def _split_hi_lo(x32):
    import ml_dtypes
    hi = x32.astype(ml_dtypes.bfloat16)
    lo = (x32 - hi.astype(np.float32)).astype(ml_dtypes.bfloat16)
    return hi, lo


def _stepv():
    sv = np.zeros((128, PHASES * 6), dtype=np.float32)
    for p in range(PHASES):
        for r in range(2):
            for i in range(3):
                sv[:, p * 6 + r * 3 + i] = (i + 1) * STEPS[p]
    return sv


def _shard_inputs(points3d_obs, points3d_pred):
    import ml_dtypes
    bf16 = ml_dtypes.bfloat16
    obs = np.asarray(points3d_obs, dtype=np.float32).reshape(BT, N_OBS, 3)
    pred = np.asarray(points3d_pred, dtype=np.float32).reshape(BT, M_PRED, 3)
    stepv = _stepv()
    in_maps = []
    for core in range(NCORES):
        so = obs[core * F:(core + 1) * F]       # [F, N, 3]
        sp = pred[core * F:(core + 1) * F]      # [F, M, 3]
        a32 = so.transpose(2, 0, 1).reshape(3, F * N_OBS)
        b32 = sp.transpose(2, 0, 1).reshape(3, F * M_PRED)
        a_hi, a_lo = _split_hi_lo(a32)
        b_hi, b_lo = _split_hi_lo(b32)
        na = (-0.5 * np.sum(a32 * a32, axis=0, dtype=np.float32))
        nb = (-0.5 * np.sum(b32 * b32, axis=0, dtype=np.float32))
        na_hi, na_lo = _split_hi_lo(na[None, :])
        nb_hi, nb_lo = _split_hi_lo(nb[None, :])
        one_a = np.ones((1, F * N_OBS), dtype=bf16)
        one_b = np.ones((1, F * M_PRED), dtype=bf16)
        obs13 = np.concatenate(
            [a_hi, a_hi, a_lo, na_hi, na_lo, one_a, one_a], axis=0)
        pred13 = np.concatenate(
            [b_hi, b_lo, b_hi, one_b, one_b, nb_hi, nb_lo], axis=0)
        in_maps.append({
            "obs_t": np.ascontiguousarray(obs13),
            "pred_t": np.ascontiguousarray(pred13),
            "stepv_t": stepv,
        })
    return in_maps


def _get_nc():
    if "nc" not in _CACHE:
        _CACHE["nc"] = _build_nc()
    return _CACHE["nc"]


def run(points3d_obs, points3d_pred, **kwargs):
    """Run on hardware; kwargs forwarded to run_bass_kernel_spmd."""
    from concourse.bass_utils import run_bass_kernel_spmd
    nc = _get_nc()
    in_maps = _shard_inputs(points3d_obs, points3d_pred)
    res = run_bass_kernel_spmd(nc, in_maps, list(range(NCORES)), **kwargs)
    return res


def kernel(points3d_obs, points3d_pred):
    res = run(points3d_obs, points3d_pred)
    loss = np.float32(res.results[0]["out"][0, 0])
    return np.asarray(loss, dtype=np.float32).reshape(())
